# revision 5
# baseline (speedup 1.0000x reference)
"""Depthwise causal conv1d kernel for Trainium2 (8 NeuronCores, SPMD).

Problem: x [B=8, T=4096, C=512] f32, weight [C=512, K=4] f32.
out[b, t, c] = sum_k weight[c, k] * x[b, t - 3 + k, c]   (causal, zero-pad)

Strategy (v4):
  - Data-parallel over batch: core b handles x[b].
  - Host-side layout: channels-first x[b].T padded with 3 leading zeros
    along time -> [C=512, T+3=4099] fp16, reshaped to [128, 4*4099]
    (4 channel chunks of 128 on partitions). fp16 halves HBM traffic;
    accumulation stays fp32 in PSUM.
  - PE computes taps 0..2 as accumulating diag-matmuls (96 x 512-col
    matmuls, 216ns each at full clock); DVE fuses tap 3 + PSUM drain +
    fp16 cast in ONE scalar_tensor_tensor per half-chunk:
    out = (x_shift3 * w3) + psum. Cuts PE time 25% vs a 4-tap design
    and eliminates the separate PSUM->SBUF copy pass.
  - The 12 diag stationary tiles are PRE-BUILT ON HOST and shipped in
    the input stream (wdiag, 384KB fp16), so PE's only dependency is
    the first half of chunk 0 -> first real matmul at ~8.6us instead
    of 13.5us (v3 built diags on-device from wcol, a longer chain).
  - A GpSimd-memset scratch feeds 8 dummy matmuls at ~6.5us to walk
    the PE clock up its pstate ramp before real data lands.
  - Sequencer roles strictly separated (a sequencer that arms DMAs
    cannot also feed compute): SP arms all inputs (arm order: c0-half0,
    wdiag, c0-half1, c1, c2, c3, wcol); ACT arms all outputs up front
    (descriptors wait in the rings on merge semaphores and burst when
    they fire); DVE only merges; PE only matmuls.
  - Output ships per half-chunk; the last half merges/ships per
    quarter to shorten the tail.
"""

import numpy as np

B, T, C, K = 8, 4096, 512, 4
P = 128  # partitions
NCHUNK = C // P  # 4 channel chunks
TJ = 512  # time-tile (free dim) per matmul; one PSUM bank
NJ = T // TJ  # 8 j-tiles per chunk
TP = T + K - 1  # padded time = 4099
TH = T // 2  # half-chunk = 2048 cols
TQ = T // 4  # quarter-chunk = 1024 cols
NW = 3 * NCHUNK  # 12 host-built diag stationaries (taps 0..2)

_compiled = None


def _build():
    import concourse.bacc as bacc
    import concourse.mybir as mybir
    from concourse.tile import TileContext

    f32 = mybir.dt.float32
    f16 = mybir.dt.float16
    nc = bacc.Bacc(enable_partition_id=False)

    wdiag_d = nc.declare_dram_parameter("wdiag", [P, NW * P], f16, isOutput=False)
    wcol_d = nc.declare_dram_parameter("wt", [P, NCHUNK * K], f32, isOutput=False)
    xw_d = nc.declare_dram_parameter("xw", [P, NCHUNK * TP], f16, isOutput=False)
    out_d = nc.declare_dram_parameter("out", [C, T], f16, isOutput=True)

    with TileContext(nc) as tc:
        with (
            tc.tile_pool(name="xpool", bufs=1) as xpool,
            tc.tile_pool(name="wpool", bufs=1) as wpool,
            tc.tile_pool(name="opool", bufs=4) as opool,
            tc.tile_pool(name="ppool", bufs=2, space="PSUM") as ppool,
        ):
            wdiag = wpool.tile([P, NW * P], f16, tag="wdiag")
            wcol = wpool.tile([P, NCHUNK * K], f32, tag="wcol")
            warm = wpool.tile([P, P], f16, tag="warm")
            xts = [
                xpool.tile([P, TP], f16, name=f"xt{c}", tag=f"xt{c}")
                for c in range(NCHUNK)
            ]

            # --- SP arms all inputs; descriptors spray over 16 rings.
            # c0's first half and the stationaries go first: PE's first
            # matmul depends only on these two transfers.
            nc.sync.dma_start(out=xts[0][:, :TH + K - 1], in_=xw_d[:, : TH + K - 1])
            nc.sync.dma_start(out=wdiag, in_=wdiag_d[:, :])
            nc.sync.dma_start(
                out=xts[0][:, TH + K - 1 :], in_=xw_d[:, TH + K - 1 : TP]
            )
            for c in range(1, NCHUNK):
                nc.sync.dma_start(out=xts[c], in_=xw_d[:, c * TP : (c + 1) * TP])
            nc.sync.dma_start(out=wcol, in_=wcol_d[:, :])

            # --- PE clock pre-warm: dummy matmuls on a GpSimd-memset tile
            # burn through the pstate ramp before real data lands
            nc.gpsimd.memset(warm, 0)
            pwarm = ppool.tile([P, TH], f32, name="pw", tag="pt")
            for i in range(8):
                nc.tensor.matmul(
                    pwarm[:, :P], warm, warm, start=True, stop=True
                )

            # --- main loop: PE 3 taps -> PSUM; DVE fuses tap3 + drain
            for c in range(NCHUNK):
                xv = xts[c]
                w3 = wcol[:, c * K + 3 : c * K + 4]
                ot = opool.tile([P, T], f16, tag="ot")
                last_chunk = c == NCHUNK - 1
                for half in range(2):
                    pt = ppool.tile([P, TH], f32, name="pt", tag="pt")
                    for j4 in range(NJ // 2):
                        j = half * (NJ // 2) + j4
                        for k in range(3):
                            nc.tensor.matmul(
                                pt[:, j4 * TJ : (j4 + 1) * TJ],
                                wdiag[:, (3 * c + k) * P : (3 * c + k + 1) * P],
                                xv[:, j * TJ + k : j * TJ + k + TJ],
                                start=(k == 0),
                                stop=(k == 2),
                            )
                    hbase = half * TH
                    tail = last_chunk and half == 1
                    nmerge = 2 if tail else 1  # quarter-merges at the tail
                    for m in range(nmerge):
                        mlo = hbase + m * (TH // nmerge)
                        mhi = mlo + TH // nmerge
                        nc.vector.scalar_tensor_tensor(
                            out=ot[:, mlo:mhi],
                            in0=xv[:, mlo + 3 : mhi + 3],
                            scalar=w3,
                            in1=pt[:, mlo - hbase : mhi - hbase],
                            op0=mybir.AluOpType.mult,
                            op1=mybir.AluOpType.add,
                        )
                        # ACT arms the output DMA; descriptors wait on the
                        # merge semaphore in the rings, bursting when it
                        # fires (so arm timing is off the critical path)
                        nc.scalar.dma_start(
                            out=out_d[c * P : (c + 1) * P, mlo:mhi],
                            in_=ot[:, mlo:mhi],
                        )

    nc.compile()
    return nc


def _prep_inputs(x: np.ndarray, weight: np.ndarray):
    # wcol[p, chunk*K + k] = weight[chunk*P + p, k]
    wcol = np.ascontiguousarray(
        weight.reshape(NCHUNK, P, K).transpose(1, 0, 2).reshape(P, NCHUNK * K)
    ).astype(np.float32)
    # wdiag[p, (3c+k)*P + m] = weight[c*P+p, k] * (m == p): diag stationaries
    wdiag = np.zeros((P, NW * P), dtype=np.float16)
    rng = np.arange(P)
    for c in range(NCHUNK):
        for k in range(3):
            wdiag[rng, (3 * c + k) * P + rng] = weight[c * P + rng, k].astype(
                np.float16
            )
    xs = []
    for b in range(B):
        xp = np.zeros((C, TP), dtype=np.float32)
        xp[:, K - 1 :] = x[b].T  # [512, 4099], 3 leading zeros
        xw = np.ascontiguousarray(
            xp.reshape(NCHUNK, P, TP).transpose(1, 0, 2).reshape(P, NCHUNK * TP)
        ).astype(np.float16)
        xs.append(xw)
    return xs, wcol, wdiag


def _in_maps(x: np.ndarray, weight: np.ndarray):
    xs, wcol, wdiag = _prep_inputs(x, weight)
    return [{"xw": xs[b], "wt": wcol, "wdiag": wdiag} for b in range(B)]


def _ensure_axon_hooks():
    """This image's antenv package lacks axon_hooks; synthesize it so a
    trace=True / BASS_TRACE run of run_bass_kernel_spmd can profile
    instead of crashing on import."""
    import sys
    import types

    if "antenv.axon_hooks" in sys.modules:
        return
    mod = types.ModuleType("antenv.axon_hooks")
    state = {"hook": None}
    mod.set_axon_ntff_profile_hook = lambda h: state.__setitem__("hook", h)
    mod.get_axon_ntff_profile_hook = lambda: state["hook"]
    sys.modules["antenv.axon_hooks"] = mod
    try:
        if "/root/.axon_site" not in sys.path:
            sys.path.insert(0, "/root/.axon_site")
        from trn_agent_boot.trn_boot import _ntff_profile_via_ctypes

        mod.set_axon_ntff_profile_hook(
            _ntff_profile_via_ctypes("/opt/axon/libaxon_pjrt.so")
        )
    except Exception:
        pass  # hook stays None; concourse degrades to no-trace


def kernel(x: np.ndarray, weight: np.ndarray) -> np.ndarray:
    global _compiled
    _ensure_axon_hooks()
    from concourse import bass_utils

    x = np.ascontiguousarray(x, dtype=np.float32)
    weight = np.ascontiguousarray(weight, dtype=np.float32)

    if _compiled is None:
        _compiled = _build()
    nc = _compiled

    in_maps = _in_maps(x, weight)
    res = bass_utils.run_bass_kernel_spmd(nc, in_maps, core_ids=list(range(B)))

    out = np.empty((B, T, C), dtype=np.float32)
    for b in range(B):
        out[b] = np.asarray(res.results[b]["out"]).astype(np.float32).T
    return out


# revision 6
# speedup vs baseline: 1.2107x; 1.2107x over previous
"""Depthwise causal conv1d kernel for Trainium2 (8 NeuronCores, SPMD).

Problem: x [B=8, T=4096, C=512] f32, weight [C=512, K=4] f32.
out[b, t, c] = sum_k weight[c, k] * x[b, t - 3 + k, c]   (causal, zero-pad)

Strategy (v5):
  - Data-parallel over batch: core b handles x[b].
  - Host-side layout: channels-first x[b].T padded with 3 leading zeros
    along time -> [C=512, T+3=4099] fp16, reshaped to [128, 4*4099]
    (4 channel chunks of 128 on partitions). fp16 halves HBM traffic;
    accumulation stays fp32 in PSUM.
  - PE computes taps 0..2 as accumulating diag-matmuls (96 x 512-col
    matmuls, 216ns each at full clock); DVE fuses tap 3 + PSUM drain +
    fp16 cast in ONE scalar_tensor_tensor per half-chunk:
    out = (x_shift3 * w3) + psum. Cuts PE time 25% vs a 4-tap design
    and eliminates the separate PSUM->SBUF copy pass.
  - The 12 diag stationary tiles are PRE-BUILT ON HOST and shipped in
    the input stream (wdiag, 384KB fp16): PE's first matmul depends
    only on {first half of chunk 0, wdiag}.
  - Chunk 0 lands as TWO tiles (3-column halo re-read from DRAM) so
    the first-half matmuls gate only on the first 0.5MB transfer.
  - wcol (the fp32 tap-3 scalars for the merge) is armed 3rd - tiny,
    so the first merge is never left waiting on it (v4's mistake:
    armed last, it landed at ~19us and stalled the whole PSUM pipe).
  - 7 x 512-col dummy matmuls on a GpSimd-memset tile walk the PE
    clock up its pstate ramp from ~7us until real data lands (~10us).
  - Sequencer roles strictly separated: SP arms all inputs; ACT arms
    all outputs up front (descriptors wait in the rings on merge
    semaphores and burst when they fire); DVE only merges; PE only
    matmuls.
  - Output ships per half-chunk; the last half merges/ships per
    quarter to shorten the tail.
"""

import numpy as np

B, T, C, K = 8, 4096, 512, 4
P = 128  # partitions
NCHUNK = C // P  # 4 channel chunks
TJ = 512  # time-tile (free dim) per matmul; one PSUM bank
NJ = T // TJ  # 8 j-tiles per chunk
TP = T + K - 1  # padded time = 4099
TH = T // 2  # half-chunk = 2048 cols
TQ = T // 4  # quarter-chunk = 1024 cols
THP = TH + K - 1  # half tile incl halo = 2051
NW = 3 * NCHUNK  # 12 host-built diag stationaries (taps 0..2)

_compiled = None


def _build():
    import concourse.bacc as bacc
    import concourse.mybir as mybir
    from concourse.tile import TileContext

    f32 = mybir.dt.float32
    f16 = mybir.dt.float16
    nc = bacc.Bacc(enable_partition_id=False)

    wdiag_d = nc.declare_dram_parameter("wdiag", [P, NW * P], f16, isOutput=False)
    wcol_d = nc.declare_dram_parameter("wt", [P, NCHUNK * K], f32, isOutput=False)
    xw_d = nc.declare_dram_parameter("xw", [P, NCHUNK * TP], f16, isOutput=False)
    out_d = nc.declare_dram_parameter("out", [C, T], f16, isOutput=True)

    with TileContext(nc) as tc:
        with (
            tc.tile_pool(name="xpool", bufs=1) as xpool,
            tc.tile_pool(name="wpool", bufs=1) as wpool,
            tc.tile_pool(name="opool", bufs=4) as opool,
            tc.tile_pool(name="ppool", bufs=2, space="PSUM") as ppool,
        ):
            wdiag = wpool.tile([P, NW * P], f16, tag="wdiag")
            wcol = wpool.tile([P, NCHUNK * K], f32, tag="wcol")
            warm = wpool.tile([P, TJ], f16, tag="warm")
            # chunk 0 split into two tiles so the first-half matmuls gate
            # only on the first transfer; chunks 1-3 arrive early enough
            # as single tiles
            xt0 = [
                xpool.tile([P, THP], f16, name=f"xt0{h}", tag=f"xt0{h}")
                for h in range(2)
            ]
            xts = [
                xpool.tile([P, TP], f16, name=f"xt{c}", tag=f"xt{c}")
                for c in range(1, NCHUNK)
            ]

            # --- SP arms all inputs; descriptors spray over 16 rings and
            # execute in arm order, so the order IS the arrival order
            nc.sync.dma_start(out=xt0[0], in_=xw_d[:, :THP])
            nc.sync.dma_start(out=wdiag, in_=wdiag_d[:, :])
            nc.sync.dma_start(out=wcol, in_=wcol_d[:, :])
            nc.sync.dma_start(out=xt0[1], in_=xw_d[:, TH : TH + THP])
            for c in range(1, NCHUNK):
                nc.sync.dma_start(
                    out=xts[c - 1], in_=xw_d[:, c * TP : (c + 1) * TP]
                )

            # --- PE clock pre-warm: dummy matmuls on a GpSimd-memset tile
            # walk the pstate ramp before real data lands
            nc.gpsimd.memset(warm, 0)
            pwarm = ppool.tile([P, TH], f32, name="pw", tag="pt")
            for i in range(7):
                nc.tensor.matmul(
                    pwarm[:, :TJ], warm[:, :P], warm, start=True, stop=True
                )

            # --- main loop: PE 3 taps -> PSUM; DVE fuses tap3 + drain
            for c in range(NCHUNK):
                w3 = wcol[:, c * K + 3 : c * K + 4]
                ot = opool.tile([P, T], f16, tag="ot")
                last_chunk = c == NCHUNK - 1
                for half in range(2):
                    # chunk 0: per-half tile, local columns; others: one tile
                    xv = xt0[half] if c == 0 else xts[c - 1]
                    vbase = 0 if c == 0 else half * TH
                    pt = ppool.tile([P, TH], f32, name="pt", tag="pt")
                    for j4 in range(NJ // 2):
                        for k in range(3):
                            lo = vbase + j4 * TJ + k
                            nc.tensor.matmul(
                                pt[:, j4 * TJ : (j4 + 1) * TJ],
                                wdiag[:, (3 * c + k) * P : (3 * c + k + 1) * P],
                                xv[:, lo : lo + TJ],
                                start=(k == 0),
                                stop=(k == 2),
                            )
                    hbase = half * TH
                    tail = last_chunk and half == 1
                    nmerge = 2 if tail else 1  # quarter-merges at the tail
                    for m in range(nmerge):
                        sz = TH // nmerge
                        plo = m * sz
                        vlo = vbase + plo + 3
                        olo = hbase + plo
                        nc.vector.scalar_tensor_tensor(
                            out=ot[:, olo : olo + sz],
                            in0=xv[:, vlo : vlo + sz],
                            scalar=w3,
                            in1=pt[:, plo : plo + sz],
                            op0=mybir.AluOpType.mult,
                            op1=mybir.AluOpType.add,
                        )
                        # ACT arms the output DMA; descriptors wait on the
                        # merge semaphore in the rings and burst when it fires
                        nc.scalar.dma_start(
                            out=out_d[c * P : (c + 1) * P, olo : olo + sz],
                            in_=ot[:, olo : olo + sz],
                        )

    nc.compile()
    return nc


def _prep_inputs(x: np.ndarray, weight: np.ndarray):
    # wcol[p, chunk*K + k] = weight[chunk*P + p, k]
    wcol = np.ascontiguousarray(
        weight.reshape(NCHUNK, P, K).transpose(1, 0, 2).reshape(P, NCHUNK * K)
    ).astype(np.float32)
    # wdiag[p, (3c+k)*P + m] = weight[c*P+p, k] * (m == p): diag stationaries
    wdiag = np.zeros((P, NW * P), dtype=np.float16)
    rng = np.arange(P)
    for c in range(NCHUNK):
        for k in range(3):
            wdiag[rng, (3 * c + k) * P + rng] = weight[c * P + rng, k].astype(
                np.float16
            )
    xs = []
    for b in range(B):
        xp = np.zeros((C, TP), dtype=np.float32)
        xp[:, K - 1 :] = x[b].T  # [512, 4099], 3 leading zeros
        xw = np.ascontiguousarray(
            xp.reshape(NCHUNK, P, TP).transpose(1, 0, 2).reshape(P, NCHUNK * TP)
        ).astype(np.float16)
        xs.append(xw)
    return xs, wcol, wdiag


def _in_maps(x: np.ndarray, weight: np.ndarray):
    xs, wcol, wdiag = _prep_inputs(x, weight)
    return [{"xw": xs[b], "wt": wcol, "wdiag": wdiag} for b in range(B)]


def _ensure_axon_hooks():
    """This image's antenv package lacks axon_hooks; synthesize it so a
    trace=True / BASS_TRACE run of run_bass_kernel_spmd can profile
    instead of crashing on import."""
    import sys
    import types

    if "antenv.axon_hooks" in sys.modules:
        return
    mod = types.ModuleType("antenv.axon_hooks")
    state = {"hook": None}
    mod.set_axon_ntff_profile_hook = lambda h: state.__setitem__("hook", h)
    mod.get_axon_ntff_profile_hook = lambda: state["hook"]
    sys.modules["antenv.axon_hooks"] = mod
    try:
        if "/root/.axon_site" not in sys.path:
            sys.path.insert(0, "/root/.axon_site")
        from trn_agent_boot.trn_boot import _ntff_profile_via_ctypes

        mod.set_axon_ntff_profile_hook(
            _ntff_profile_via_ctypes("/opt/axon/libaxon_pjrt.so")
        )
    except Exception:
        pass  # hook stays None; concourse degrades to no-trace


def kernel(x: np.ndarray, weight: np.ndarray) -> np.ndarray:
    global _compiled
    _ensure_axon_hooks()
    from concourse import bass_utils

    x = np.ascontiguousarray(x, dtype=np.float32)
    weight = np.ascontiguousarray(weight, dtype=np.float32)

    if _compiled is None:
        _compiled = _build()
    nc = _compiled

    in_maps = _in_maps(x, weight)
    res = bass_utils.run_bass_kernel_spmd(nc, in_maps, core_ids=list(range(B)))

    out = np.empty((B, T, C), dtype=np.float32)
    for b in range(B):
        out[b] = np.asarray(res.results[b]["out"]).astype(np.float32).T
    return out
